# revision 5
# baseline (speedup 1.0000x reference)
"""Gumbel top-k sampler for Trainium2 (Bass/Tile), 8-core data parallel. v2.

Math (per row, vocab V=50257, k=50):
    g    = logits - ln(-ln(u + eps) + eps)
    t    ~= k-th largest of g        (threshold)
    mask = sigmoid(g - t)
    out  = softmax(logits * mask)

v2 design (vs the topk-instruction baseline at 2.80 ms):
  * 16-bit I/O: logits as fp16, u as uint16 fixed point (dequant
    (u16+0.5)/2^16 folded into the first Ln's scale/bias), output as fp16
    scaled by 1024 (un-scaled on host). Halves HBM traffic.
  * Threshold via Newton iteration on the W=8 block-maxima instead of the
    GPSIMD topk instruction (which costs ~50us/tile):
      - mx = max over adjacent 8 elements of g  (DVE reduce, [128,393])
      - 4x Newton: cnt = #{mx > t} (DVE tensor_scalar is_gt + accum),
        row-sum of 16 partitions (PE m16 matmul), t += damp*ln(cnt/k)
        (ACT Ln, same table set as the main Ln/Exp passes).
    The k-th largest block-max equals the k-th largest element unless two
    top-k elements share a block (~17% of rows, threshold then off by one
    rank gap). Measured end-to-end l2 rel err ~1.4e-3.
  * All ACT functions except Sigmoid live in natural_log_exp_and_others;
    emission order keeps the per-group ACT stream [Ln..][Sig..][Exp..]
    [newton Ln..] so walrus inserts only 2 table loads per group.
  * Newton/normalize scalar work is batched per group into [128, G] tiles
    (one Ln / matmul / STT per group-iteration instead of per tile).
  * fp16 DVE ops run at 2x (tensor_tensor) / 4x (tensor_scalar).
  * GPSIMD: unused. PE: tiny row-sum/broadcast matmuls.
"""

import numpy as np

import concourse.bass as bass
import concourse.bacc as bacc
import concourse.tile as tile
from concourse import mybir
from concourse.bass_utils import run_bass_kernel_spmd

F32 = mybir.dt.float32
F16 = mybir.dt.float16
U16 = mybir.dt.uint16
AF = mybir.ActivationFunctionType
ALU = mybir.AluOpType

B, V = 2048, 50257
NCORES = 8
ROWS = B // NCORES            # 256 rows per core
TOK = 8                       # rows per tile
NPART = 128
VPAD = 50304                  # 16 * 3144
CHUNK = VPAD // 16            # 3144 elements per partition
NTILES = ROWS // TOK          # 32 tiles per core
GROUP = 4                     # tiles per pipeline group

EPS = 1e-10
USCALE = 1.0 / 65536.0        # u dequant scale
UBIAS = 0.5 / 65536.0 + EPS   # u dequant half-offset + eps
NEWTON_ITERS = 3
LNBIAS = 0.01                 # ln clamp: ln(cnt/k + 0.01) bounds t-step at -4.6
NEWTON_T0 = 5.5               # init threshold (randn logits + gumbel, k=50)
DAMP_LAST = 0.7               # damping on final Newton step
W = 8                         # block-max width
NBLK = CHUNK // W             # 393 block maxima per partition
OUT_SCALE = 1024.0            # fp16 output scale (keeps probs in normal range)

# pads: logits=0, u16=24109 -> u~1/e -> noise~0 -> g_pad ~ 0, far below
# threshold (~5.5); masked_pad = 0 exactly so each pad adds exp(0)=1 to its
# row's softmax sum; subtract NPADS from Z.
PAD_U16 = 24109
NPADS = VPAD - V              # 47


def _build_program(k: int, ntiles: int = NTILES):
    assert 1 <= k <= 1000
    nc = bacc.Bacc("TRN2", target_bir_lowering=False, debug=False)

    # activation float biases must exist as [128,1] const APs in SBUF
    for cval in (EPS, UBIAS, LNBIAS):
        ct = nc.alloc_sbuf_tensor(f"const-float32-{cval}", [128, 1], F32)
        nc.gpsimd.memset(ct.ap(), cval)
        nc.const_aps.aps[(F32, cval)] = ct.ap()
    nc.all_engine_barrier()

    l_dram = nc.dram_tensor("logits", [ntiles * TOK * VPAD], F16,
                            kind="ExternalInput")
    u_dram = nc.dram_tensor("u", [ntiles * TOK * VPAD], U16,
                            kind="ExternalInput")
    # 16x16 block-diagonal ones: row-sum + broadcast over each row's 16
    # partitions in one matmul
    m16_dram = nc.dram_tensor("m16", [NPART, NPART], F32, kind="ExternalInput")
    o_dram = nc.dram_tensor("out", [ntiles * TOK, VPAD], F16,
                            kind="ExternalOutput")

    from contextlib import ExitStack
    with tile.TileContext(nc) as tc, ExitStack() as es:
        consts = es.enter_context(tc.tile_pool(name="consts", bufs=1))
        lpool = es.enter_context(tc.tile_pool(name="lpool", bufs=3 * GROUP + 1))
        upool = es.enter_context(tc.tile_pool(name="upool", bufs=3 * GROUP + 1))
        mxpool = es.enter_context(tc.tile_pool(name="mxpool", bufs=2 * GROUP + 2))
        scpool = es.enter_context(tc.tile_pool(name="scpool", bufs=8))
        gsm = es.enter_context(tc.tile_pool(name="gsm", bufs=14))
        psum = es.enter_context(tc.tile_pool(name="psum", bufs=4, space="PSUM"))

        m16 = consts.tile([NPART, NPART], F32, tag="m16")
        nc.sync.dma_start(m16[:], m16_dram.ap())
        t0 = consts.tile([NPART, 1], F32, tag="t0")
        nc.vector.memset(t0[:], NEWTON_T0)
        t0g = consts.tile([NPART, GROUP], F32, tag="t0g")
        nc.vector.memset(t0g[:], NEWTON_T0)

        def in_ap(handle, i):
            # contiguous [128 partitions, 3144] view of padded rows 8i..8i+7
            return bass.AP(handle, i * TOK * VPAD,
                           [[CHUNK, NPART], [1, CHUNK]])

        state = {}
        gstate = {}

        def p1_ld(i):
            """load + gumbel noise (DMA and ACT only).

            ut's buffer is reused through the whole tile lifetime:
            u16 -> noise -> g -> mask -> masked -> exp -> scaled out."""
            lt = lpool.tile([NPART, CHUNK], F16, tag="lt")
            ut = upool.tile([NPART, CHUNK], U16, tag="ut")
            nc.sync.dma_start(lt[:], in_ap(l_dram, i))
            nc.sync.dma_start(ut[:], in_ap(u_dram, i))
            uf = ut[:].bitcast(F16)
            # x1 = ln((u16+0.5)/65536 + eps); noise' = ln(-x1 + eps)
            nc.scalar.activation(uf, ut[:], AF.Ln, bias=UBIAS, scale=USCALE)
            nc.scalar.activation(uf, uf, AF.Ln, bias=EPS, scale=-1.0)
            state[i] = {"lt": lt, "ut": ut, "uf": uf}

        def p1_dve(i):
            """g = logits - noise (in place), then W=8 block maxima"""
            st_ = state[i]
            mx = mxpool.tile([NPART, NBLK], F16, tag="mx")
            nc.vector.tensor_sub(st_["uf"], st_["lt"][:], st_["uf"])
            nc.vector.reduce_max(
                mx[:], st_["uf"].rearrange("p (b w) -> p b w", w=W),
                axis=mybir.AxisListType.X)
            st_["mx"] = mx

        def p2_count(gi, grp, it):
            """batched Newton count for a whole group: cnt_j = #{mx_j > t_j}"""
            gs = gstate[gi]
            G = len(grp)
            cng = gsm.tile([NPART, G], F32, tag="cng")
            for j, i in enumerate(grp):
                t_ap = t0[:] if it == 0 else gs["t"][:, j:j + 1]
                csc = scpool.tile([NPART, NBLK], F16, tag="csc")
                nc.vector.tensor_scalar(csc[:], state[i]["mx"][:], t_ap, None,
                                        ALU.is_gt, op1=ALU.add,
                                        accum_out=cng[:, j:j + 1])
            c16 = psum.tile([NPART, G], F32, tag="c16")
            nc.tensor.matmul(c16[:], m16[:], cng[:], start=True, stop=True)
            lc = gsm.tile([NPART, G], F32, tag="lc")
            nc.scalar.activation(lc[:], c16[:], AF.Ln, bias=LNBIAS,
                                 scale=1.0 / k)
            gs["lc"] = lc

        def p2_update(gi, grp, it):
            """t += damp * ln(cnt/k); final iteration produces thn = -t"""
            gs = gstate[gi]
            G = len(grp)
            lc = gs.pop("lc")
            tn = gsm.tile([NPART, G], F32, tag="tn")
            if it == 0:
                nc.vector.scalar_tensor_tensor(tn[:], lc[:], 1.0,
                                               gs["t0g"][:, :G],
                                               ALU.mult, ALU.add)
            elif it < NEWTON_ITERS - 1:
                nc.vector.scalar_tensor_tensor(tn[:], lc[:], 1.0, gs["t"][:],
                                               ALU.mult, ALU.add)
            else:
                # thn = -(t + damp*lc) = (lc * -damp) - t, the sigmoid bias
                nc.vector.scalar_tensor_tensor(tn[:], lc[:], -DAMP_LAST,
                                               gs["t"][:], ALU.mult,
                                               ALU.subtract)
                gs["thn"] = tn
            gs["t"] = tn

        def p3_sig(gi, grp):
            gs = gstate[gi]
            for j, i in enumerate(grp):
                st_ = state[i]
                nc.scalar.activation(st_["uf"], st_["uf"], AF.Sigmoid,
                                     bias=gs["thn"][:, j:j + 1])

        def p3_mul(grp):
            for i in grp:
                st_ = state[i]
                # masked = mask * logits, in place; logits dead after this
                nc.vector.tensor_mul(st_["uf"], st_["uf"], st_["lt"][:])

        def p3_exp(gi, grp):
            gs = gstate[gi]
            sumg = gsm.tile([NPART, len(grp)], F32, tag="sumg")
            for j, i in enumerate(grp):
                st_ = state[i]
                nc.scalar.activation(st_["uf"], st_["uf"], AF.Exp,
                                     accum_out=sumg[:, j:j + 1])
            gs["sumg"] = sumg

        def p3_out(gi, grp):
            gs = gstate[gi]
            G = len(grp)
            z16 = psum.tile([NPART, G], F32, tag="z16")
            nc.tensor.matmul(z16[:], m16[:], gs["sumg"][:], start=True,
                             stop=True)
            zc = gsm.tile([NPART, G], F32, tag="zc")
            # Z = z16 - NPADS, pre-divided by OUT_SCALE so recip gives 1024/Z
            nc.vector.tensor_scalar(zc[:], z16[:], -float(NPADS),
                                    1.0 / OUT_SCALE, ALU.add, op1=ALU.mult)
            rz = gsm.tile([NPART, G], F32, tag="rz")
            nc.vector.reciprocal(rz[:], zc[:])
            for j, i in enumerate(grp):
                st_ = state.pop(i)
                nc.vector.tensor_scalar_mul(st_["uf"], st_["uf"],
                                            rz[:, j:j + 1])
                out_view = o_dram.ap()[i * TOK:(i + 1) * TOK, :].rearrange(
                    "r (c e) -> r c e", e=CHUNK)
                nc.sync.dma_start(out_view, st_["uf"])

        groups = [list(range(g, min(g + GROUP, ntiles)))
                  for g in range(0, ntiles, GROUP)]
        if len(groups[-1]) == GROUP and GROUP >= 4:
            # split the last group so the pipeline drain exposes less work
            tail = groups.pop()
            h = GROUP // 2
            groups += [tail[:h], tail[h:]]

        def p3_all(gi, grp):
            p3_sig(gi, grp)
            p3_mul(grp)
            p3_exp(gi, grp)
            p3_out(gi, grp)

        # 3-deep software pipeline: step s runs p1(s) | newton iters 0-1 of
        # s-1, iters 2-3 of s-2 | mask/exp/out of s-2. Each newton op is
        # emitted where its 4-hop (DVE count -> PE sum -> ACT ln -> DVE
        # update) chain has a phase of independent work to hide behind, and
        # newton Lns land inside existing Ln runs / at the Sigmoid-set
        # boundary so table switches stay at 2 per step.
        def emit_step(s, ngroups):
            grp = groups[s] if s < ngroups else None
            pa = s - 1 if 0 <= s - 1 < ngroups else None   # newton iters 0-1
            pb = s - 2 if 0 <= s - 2 < ngroups else None   # newton 2-3 + out
            # Newton counts whose inputs are a full step old are emitted at
            # the head of the step's DVE stream (ready immediately), ahead of
            # the bulky subs/reduces, so each iteration's Ln input is ready
            # by the time ACT reaches it inside the p1 Ln run. p1's DVE work
            # is interleaved afterwards, pacing the chain.
            if grp is not None:
                gstate[s] = {"t0g": t0g}
                for i in grp:
                    p1_ld(i)
            if pa is not None:
                p2_count(pa, groups[pa], 0)
            if pb is not None:
                p2_count(pb, groups[pb], 2)   # final iter -> thn
            if grp is not None:
                p1_dve(grp[0])
            if pa is not None:
                p2_update(pa, groups[pa], 0)
            if pb is not None:
                p2_update(pb, groups[pb], 2)
            if pa is not None:
                p2_count(pa, groups[pa], 1)
            if grp is not None:
                p1_dve(grp[1])
            if pa is not None:
                p2_update(pa, groups[pa], 1)
            if pb is not None:
                p3_sig(pb, groups[pb])
            if grp is not None:
                for i in grp[2:]:
                    p1_dve(i)
            if pb is not None:
                p3_mul(groups[pb])
                p3_exp(pb, groups[pb])
                p3_out(pb, groups[pb])

        ng = len(groups)
        for s in range(ng + 2):
            emit_step(s, ng)

    nc.compile()
    return nc


def _m16():
    m16 = np.zeros((NPART, NPART), np.float32)
    for p in range(NPART):
        g = (p // 16) * 16
        m16[g:g + 16, p] = 1.0
    return m16


def _core_inputs(l16, u16, c):
    sl = slice(c * ROWS, (c + 1) * ROWS)
    lp = np.zeros((ROWS, VPAD), np.float16)
    lp[:, :V] = l16[sl]
    up = np.full((ROWS, VPAD), PAD_U16, np.uint16)
    up[:, :V] = u16[sl]
    return {"logits": lp.reshape(-1), "u": up.reshape(-1), "m16": _m16()}


_PROGRAM_CACHE = {}


def _program(k: int):
    if k not in _PROGRAM_CACHE:
        _PROGRAM_CACHE[k] = _build_program(k)
    return _PROGRAM_CACHE[k]


def _ensure_ntff_hook():
    """This image's antenv lacks axon_hooks; recreate it with the boot
    script's ctypes NTFF hook so trace=True works."""
    import sys
    import types
    try:
        import antenv.axon_hooks  # noqa: F401
        return
    except ImportError:
        pass
    import antenv
    mod = types.ModuleType("antenv.axon_hooks")
    _h = [None]
    mod.set_axon_ntff_profile_hook = lambda hook: _h.__setitem__(0, hook)
    mod.get_axon_ntff_profile_hook = lambda: _h[0]
    sys.modules["antenv.axon_hooks"] = mod
    antenv.axon_hooks = mod
    try:
        from trn_agent_boot.trn_boot import _ntff_profile_via_ctypes
        mod.set_axon_ntff_profile_hook(
            _ntff_profile_via_ctypes("/opt/axon/libaxon_pjrt.so"))
    except Exception:
        pass


def kernel(logits: np.ndarray, u: np.ndarray, k, _trace: bool = False):
    k = int(np.asarray(k))
    if _trace:
        _ensure_ntff_hook()
    logits = np.ascontiguousarray(logits, dtype=np.float32)
    u = np.ascontiguousarray(u, dtype=np.float32)
    assert logits.shape == (B, V) and u.shape == (B, V)

    l16 = logits.astype(np.float16)
    u16 = (u * 65536.0).astype(np.uint16)  # floor; dequant adds half-offset

    nc = _program(k)
    in_maps = [_core_inputs(l16, u16, c) for c in range(NCORES)]

    res = run_bass_kernel_spmd(nc, in_maps, core_ids=list(range(NCORES)),
                               trace=_trace)
    out = np.empty((B, V), np.float32)
    inv = 1.0 / OUT_SCALE
    for c in range(NCORES):
        out[c * ROWS:(c + 1) * ROWS] = (
            res.results[c]["out"][:, :V].astype(np.float32) * inv)
    if _trace:
        return out, res
    return out
